# revision 5
# baseline (speedup 1.0000x reference)
"""Multi-head attention (B=4, S=2048, D=1024, H=16, causal) on 8 TRN2 NeuronCores.

Sharding: batch (4) x head-group (2 groups of 8 heads) = 8 cores.
Megatron-style: wq/wk/wv column-parallel, wo row-parallel; the 2-way partial-sum
of the row-parallel output projection is folded into the host-side unshard.

Per-core algorithm (heads h in the core's group, s-chunks of 512 queries):
  QT[dk, s], KT[dk, s] = (x @ w + b)^T via PE matmuls on host-pre-transposed
  inputs; V[s, dv] likewise, with a ones-column appended per head so that the
  PV matmul also produces softmax denominators.
  scoresT[k, q] = KT^T-slices x QT (two heads packed in the 128-partition dim,
  concurrent via PE row tiling since dk=64).
  E = exp(scoresT/8) on ACT (no max-subtraction needed: scores ~ N(0,1));
  causal masking multiplies the 4 diagonal-crossing blocks per q-chunk by
  precomputed 0/1 tiles; fully-masked blocks are never computed.
  ctxT[dv, q] accumulates V^T-slices x E in PSUM; row 64 = sum(E).
  Normalize: reciprocal (DVE) -> partition_broadcast (GPSIMD) -> multiply (DVE).
  y_partial[s, do] = sum over head-pairs of ctxT-slices x wo-rows (PSUM accum).

All matmul operands are float32r (TF32-like, full PE rate, ~1.4e-4 rounding).
"""
import sys
import numpy as np

sys.path.insert(0, "/opt/trn_rl_repo")

from contextlib import ExitStack

import concourse.bacc as bacc
import concourse.tile as tile
from concourse import mybir
from concourse.bass_utils import run_bass_kernel_spmd

F32 = mybir.dt.float32
F32R = mybir.dt.float32r

B, S, D, H = 4, 2048, 1024, 16
DK = D // H          # 64
HG = H // 2          # 8 heads per core
DG = HG * DK         # 512 columns per core group
SC = 512             # query-chunk width
KB = 128             # key-block height
N_SC = S // SC       # 4
N_KB = S // KB       # 16
N_DM = D // 128      # 8 contraction tiles for projections
N_PAIR = HG // 2     # 4 head pairs per core
EXPSCALE = 1.0 / 8.0  # 1/sqrt(DK)


def build_program():
    """Emit the SPMD Bass program (identical on all 8 cores)."""
    nc = bacc.Bacc("TRN2", target_bir_lowering=False, debug=False)

    qT_in = nc.dram_tensor("qT", [D, S], F32R, kind="ExternalInput").ap()
    kT_in = nc.dram_tensor("kT", [D, S], F32R, kind="ExternalInput").ap()
    vT_in = nc.dram_tensor("vT", [D, S], F32R, kind="ExternalInput").ap()
    wq_in = nc.dram_tensor("wq", [D, DG], F32R, kind="ExternalInput").ap()
    wk_in = nc.dram_tensor("wk", [D, DG], F32R, kind="ExternalInput").ap()
    wv_in = nc.dram_tensor("wv", [D, DG], F32R, kind="ExternalInput").ap()
    wo_in = nc.dram_tensor("wo", [DG, D], F32R, kind="ExternalInput").ap()
    bq_in = nc.dram_tensor("bq", [1, DG], F32R, kind="ExternalInput").ap()
    bk_in = nc.dram_tensor("bk", [1, DG], F32R, kind="ExternalInput").ap()
    bv_in = nc.dram_tensor("bv", [1, DG], F32R, kind="ExternalInput").ap()
    ones_in = nc.dram_tensor("ones", [1, SC], F32R, kind="ExternalInput").ap()
    vones_in = nc.dram_tensor("vones", [128, HG], F32R, kind="ExternalInput").ap()
    masks_in = nc.dram_tensor("masks", [4, KB, SC], F32R, kind="ExternalInput").ap()
    y_out = nc.dram_tensor("y", [S, D], F32, kind="ExternalOutput").ap()

    with tile.TileContext(nc) as tc, ExitStack() as ctx:
        stage = ctx.enter_context(tc.tile_pool(name="stage", bufs=8))
        wpool = ctx.enter_context(tc.tile_pool(name="wpool", bufs=8))
        wopool = ctx.enter_context(tc.tile_pool(name="wopool", bufs=1))
        qkpool = ctx.enter_context(tc.tile_pool(name="qkpool", bufs=1))
        vpool = ctx.enter_context(tc.tile_pool(name="vpool", bufs=1))
        epool = ctx.enter_context(tc.tile_pool(name="epool", bufs=2))
        cpool = ctx.enter_context(tc.tile_pool(name="cpool", bufs=6))
        mpool = ctx.enter_context(tc.tile_pool(name="mpool", bufs=1))
        ypool = ctx.enter_context(tc.tile_pool(name="ypool", bufs=2))
        rpool = ctx.enter_context(tc.tile_pool(name="rpool", bufs=2))
        bpool = ctx.enter_context(tc.tile_pool(name="bpool", bufs=2))
        onepool = ctx.enter_context(tc.tile_pool(name="onepool", bufs=1))
        pspool = ctx.enter_context(tc.tile_pool(name="pspool", bufs=1, space="PSUM"))

        # ---- constants ----
        ones_sb = onepool.tile([1, SC], F32R, name="ones_sb")
        nc.sync.dma_start(ones_sb[:], ones_in[:])
        bq_sb = onepool.tile([1, DG], F32R, name="bq_sb")
        nc.sync.dma_start(bq_sb[:], bq_in[:])
        bk_sb = onepool.tile([1, DG], F32R, name="bk_sb")
        nc.sync.dma_start(bk_sb[:], bk_in[:])
        bv_sb = onepool.tile([1, DG], F32R, name="bv_sb")
        nc.sync.dma_start(bv_sb[:], bv_in[:])
        masks_sb = []
        for j in range(4):
            m = mpool.tile([KB, SC], F32R, name=f"mask{j}")
            nc.sync.dma_start(m[:], masks_in[j])
            masks_sb.append(m)

        # ---- output regions of the projections ----
        # QT/KT: per head-pair tile [128, S]; rows 0:64 head 2p, 64:128 head 2p+1.
        QT = [qkpool.tile([128, S], F32R, name=f"QT{p}") for p in range(N_PAIR)]
        KT = [qkpool.tile([128, S], F32R, name=f"KT{p}") for p in range(N_PAIR)]
        # V: per key-block tile [128, HG*65]; per head 64 value cols + ones col.
        V = [vpool.tile([128, HG, 65], F32R, name=f"V{kb}") for kb in range(N_KB)]
        for kb in range(N_KB):
            nc.sync.dma_start(V[kb][:, :, 64:65], vones_in.unsqueeze(-1))

        # ---- projections ----
        w_sb = {}
        for nm, w_in in (("wq", wq_in), ("wk", wk_in), ("wv", wv_in)):
            w_sb[nm] = []
            for dm in range(N_DM):
                t = wpool.tile([128, DG], F32R, name=f"{nm}_{dm}", tag="w")
                nc.sync.dma_start(t[:], w_in[dm * 128:(dm + 1) * 128, :])
                w_sb[nm].append(t)
        wo_sb = []
        for p in range(N_PAIR):
            t = wopool.tile([128, D], F32R, name=f"wo_{p}")
            nc.sync.dma_start(t[:], wo_in[p * 128:(p + 1) * 128, :])
            wo_sb.append(t)

        for nm, xT_in, bias, dst in (
            ("q", qT_in, bq_sb, QT),
            ("k", kT_in, bk_sb, KT),
            ("v", vT_in, bv_sb, V),
        ):
            for sc in range(N_SC):
                xs = []
                for dm in range(N_DM):
                    t = stage.tile([128, SC], F32R, name=f"{nm}s{sc}_{dm}", tag="stage")
                    nc.sync.dma_start(
                        t[:], xT_in[dm * 128:(dm + 1) * 128, sc * SC:(sc + 1) * SC]
                    )
                    xs.append(t)
                if nm != "v":
                    # out[dk, s] accumulated over dm; init row = bias broadcast.
                    for p in range(N_PAIR):
                        ps = pspool.tile([128, SC], F32, name=f"ps_{nm}", tag="psa",
                                         bufs=2)
                        nc.tensor.matmul(
                            ps[:], bias[0:1, p * 128:(p + 1) * 128], ones_sb[0:1, :],
                            start=True, stop=False,
                        )
                        for dm in range(N_DM):
                            nc.tensor.matmul(
                                ps[:],
                                w_sb["w" + nm][dm][:, p * 128:(p + 1) * 128],
                                xs[dm][:],
                                start=False, stop=(dm == N_DM - 1),
                            )
                        nc.vector.tensor_copy(
                            dst[p][:, sc * SC:(sc + 1) * SC], ps[:]
                        )
                else:
                    # out[s, dv] per 128-row s-block; init row = bv broadcast.
                    for sb in range(4):
                        ps = pspool.tile([128, DG], F32, name="ps_v", tag="psa",
                                         bufs=2)
                        nc.tensor.matmul(
                            ps[:], ones_sb[0:1, 0:128], bias[0:1, :],
                            start=True, stop=False,
                        )
                        for dm in range(N_DM):
                            nc.tensor.matmul(
                                ps[:],
                                xs[dm][:, sb * 128:(sb + 1) * 128],
                                w_sb["wv"][dm][:],
                                start=False, stop=(dm == N_DM - 1),
                            )
                        kb = sc * 4 + sb
                        nc.vector.tensor_copy(
                            V[kb][:, :, 0:64], ps[:].rearrange("p (h d) -> p h d", h=HG)
                        )

        # ---- attention + output projection, q-chunk outer ----
        for qc in range(N_SC):
            kbmax = 4 * (qc + 1)
            ctx_pairs = []
            for p in range(N_PAIR):
                h0, h1 = 2 * p, 2 * p + 1
                ctx0 = pspool.tile([65, SC], F32, name="ctx0", tag="psctx0", bufs=1)
                ctx1 = pspool.tile([65, SC], F32, name="ctx1", tag="psctx1", bufs=1)
                for kb in range(kbmax):
                    scps = pspool.tile([128, 2 * SC], F32, name="scps", tag="pssc",
                                       bufs=2)
                    # scoresT: two heads concurrently via PE row groups
                    nc.tensor.matmul(
                        scps[:, 0:SC],
                        KT[p][0:64, kb * KB:(kb + 1) * KB],
                        QT[p][0:64, qc * SC:(qc + 1) * SC],
                        start=True, stop=True,
                    )
                    nc.tensor.matmul(
                        scps[:, SC:2 * SC],
                        KT[p][64:128, kb * KB:(kb + 1) * KB],
                        QT[p][64:128, qc * SC:(qc + 1) * SC],
                        start=True, stop=True,
                    )
                    e = epool.tile([128, 2 * SC], F32R, name="e", tag="e")
                    nc.scalar.activation(
                        e[:], scps[:], mybir.ActivationFunctionType.Exp,
                        scale=EXPSCALE,
                    )
                    j = kb - 4 * qc
                    if j >= 0:  # diagonal-crossing block: apply causal mask
                        nc.vector.tensor_mul(e[:, 0:SC], e[:, 0:SC], masks_sb[j][:])
                        nc.vector.tensor_mul(e[:, SC:2 * SC], e[:, SC:2 * SC],
                                             masks_sb[j][:])
                    first, last = kb == 0, kb == kbmax - 1
                    nc.tensor.matmul(
                        ctx0[:], V[kb][:, h0, :], e[:, 0:SC],
                        start=first, stop=last,
                    )
                    nc.tensor.matmul(
                        ctx1[:], V[kb][:, h1, :], e[:, SC:2 * SC],
                        start=first, stop=last,
                    )
                # normalize: ctx rows 0:64 / ctx row 64
                cp = cpool.tile([128, SC], F32R, name="cp", tag="ctx")
                for i, cps in ((0, ctx0), (1, ctx1)):
                    rec = rpool.tile([1, SC], F32, name="rec", tag="rec", bufs=4)
                    nc.vector.reciprocal(rec[:], cps[64:65, :])
                    rb = bpool.tile([64, SC], F32, name="rb", tag="rb", bufs=4)
                    nc.gpsimd.partition_broadcast(rb[:], rec[:])
                    nc.vector.tensor_tensor(
                        cp[i * 64:(i + 1) * 64, :], cps[0:64, :], rb[:],
                        mybir.AluOpType.mult,
                    )
                ctx_pairs.append(cp)

            # output projection for this q-chunk
            for sb in range(4):
                yst = ypool.tile([128, D], F32, name="yst", tag="y")
                for dc in range(2):
                    yps = pspool.tile([128, SC], F32, name="yps", tag="psa", bufs=2)
                    for p in range(N_PAIR):
                        nc.tensor.matmul(
                            yps[:],
                            ctx_pairs[p][:, sb * 128:(sb + 1) * 128],
                            wo_sb[p][:, dc * SC:(dc + 1) * SC],
                            start=(p == 0), stop=(p == N_PAIR - 1),
                        )
                    nc.vector.tensor_copy(yst[:, dc * SC:(dc + 1) * SC], yps[:])
                row = qc * SC + sb * 128
                nc.sync.dma_start(y_out[row:row + 128, :], yst[:])

    nc.compile()
    return nc


def make_inputs(q, k, v, wq, bq, wk, bk, wv, bv, wo):
    """Host-side shard + layout prep. Returns list of 8 per-core input dicts."""
    f = np.float32
    masks = np.zeros((4, KB, SC), f)
    qj = np.arange(SC)[None, :]
    ki = np.arange(KB)[:, None]
    for j in range(4):
        masks[j] = (qj >= j * KB + ki).astype(f)
    ones = np.ones((1, SC), f)
    vones = np.ones((128, HG), f)

    qT = [np.ascontiguousarray(np.asarray(q[b]).T) for b in range(B)]
    kT = [np.ascontiguousarray(np.asarray(k[b]).T) for b in range(B)]
    vT = [np.ascontiguousarray(np.asarray(v[b]).T) for b in range(B)]

    in_maps = []
    for c in range(8):
        b, g = c // 2, c % 2
        sl = slice(g * DG, (g + 1) * DG)
        in_maps.append({
            "qT": qT[b], "kT": kT[b], "vT": vT[b],
            "wq": np.ascontiguousarray(wq[:, sl]),
            "wk": np.ascontiguousarray(wk[:, sl]),
            "wv": np.ascontiguousarray(wv[:, sl]),
            "wo": np.ascontiguousarray(wo[sl, :]),
            "bq": np.ascontiguousarray(bq[sl]).reshape(1, DG),
            "bk": np.ascontiguousarray(bk[sl]).reshape(1, DG),
            "bv": np.ascontiguousarray(bv[sl]).reshape(1, DG),
            "ones": ones, "vones": vones, "masks": masks,
        })
    return in_maps


def combine_outputs(results, bo):
    """Sum the two row-parallel partials per batch and add the output bias."""
    out = np.empty((B, S, D), np.float32)
    for b in range(B):
        out[b] = results[2 * b]["y"] + results[2 * b + 1]["y"] + np.asarray(bo)[None, :]
    return out


_NC_CACHE = {}


def kernel(x, q, k, v, mask, wq, bq, wk, bk, wv, bv, wo, bo):
    # x is unused (overwritten in the reference forward); mask is the causal
    # tril mask, which is hardcoded in the on-device masking.
    if "nc" not in _NC_CACHE:
        _NC_CACHE["nc"] = build_program()
    nc = _NC_CACHE["nc"]
    in_maps = make_inputs(q, k, v, wq, bq, wk, bk, wv, bv, wo)
    r = run_bass_kernel_spmd(nc, in_maps, core_ids=list(range(8)))
    return combine_outputs(r.results, bo)
